# revision 21
# baseline (speedup 1.0000x reference)
"""CharRNN (2-layer LSTM + softmax CE) Trainium2 Bass kernel.

Sharding: data-parallel over batch (B=64 -> 8 rows/core on 8 cores).
Each core runs the full T=128 recurrence for its 8 sequences and the
cross-entropy over its own 1024 tokens; host sums the 8 partial NLLs
(final per-token ln() + reduction also on host — 8KB/core of output).

Device design (per core):
  - Interleaved cells: cell0(step s) at partitions 0:8 and cell1(step
    s-1) at partitions 32:40 share every instruction (distinct PE column
    groups let their matmuls overlap on the array; ACT/DVE cost is
    free-dim bound so the packing halves instruction count).
  - All gates via tanh only (sigmoid(x) = (1+tanh(x/2))/2, with the /2
    folded into the i/o/f weight columns on the host and the cell state
    kept as c' = 2c). One activation table set (exp_and_others holds
    tanh AND exp) so cross-entropy work interleaves with the recurrence
    with no table thrash.
  - z = [x,h] @ W with x_h^T as stationary operand and W streamed
    (weights re-stream each step; batch is tiny so this is the cheap
    direction). The x@Wx0+b0 part is batched over all tokens up front
    into an SBUF buffer and injected per-step with an identity-selector
    matmul (halves the per-step x-part stream).
  - h transposed each step via identity matmul (PSUM junk rows are
    zeroed once: NaN junk would poison the product since NaN*0=NaN).
  - CE: softmax_w lives in SBUF (8MB bf16); logits [128 tok, 500 vocab]
    chunks are computed/exp'ed/summed *inside* the step loop (2-3 chunks
    per step once their tokens exist) to fill PE gaps and keep the HAM
    clock warm. Target logit via indirect row gather of host-transposed
    softmax_w and a multiply+reduce.
"""

import sys

for _p in ("/opt/trn_rl_repo",):
    if _p not in sys.path:
        sys.path.insert(0, _p)

import ml_dtypes
import numpy as np

import concourse.bass as bass
import concourse.mybir as mybir
import concourse.tile as tile
from concourse.bass_utils import run_bass_kernel_spmd
from concourse.masks import make_identity

F32 = mybir.dt.float32
BF16 = mybir.dt.bfloat16
I32 = mybir.dt.int32
AF = mybir.ActivationFunctionType
ALU = mybir.AluOpType
AX = mybir.AxisListType

# Problem shapes (hardcoded per contest rules).
V, B, T, U = 16000, 64, 128, 256
NCORES = 8
BL = B // NCORES            # 8 batch rows per core
NTOK = BL * T               # 1024 tokens per core
G4 = 4 * U                  # 1024 gate width
NG = 8                      # embedding gathers per core (128 tokens each)
VC = 500                    # vocab chunk for CE
NVC = V // VC               # 32 chunks
NTT = NTOK // 128           # 8 token tiles for CE

# Gate permutation: reference z columns are [i|j|f|o]; we reorder to
# [i|o|f|j]. Slices in permuted space (all of i,o,f get tanh(x/2) with
# the 1/2 folded into the weights; j gets plain tanh):
_TI = slice(0 * U, 1 * U)
_TO = slice(1 * U, 2 * U)
_TF = slice(2 * U, 3 * U)
_TJ = slice(3 * U, 4 * U)
GPERM = np.r_[0:U, 3 * U:4 * U, 2 * U:3 * U, U:2 * U]

FORGET_BIAS = 1.0

_CACHE = {}


def _split_multiwaits(nc):
    """Walrus codegen supports only ONE semaphore wait per HW instruction
    (NEURON_ISA_TPB_EVENTS has a single wait slot) and errors out on
    instructions carrying more. Tile emits multi-wait sync_info freely, so
    split: for an instruction with k>1 waits, insert k-1 NoOps on the same
    engine queue immediately before it, each carrying one wait. Identical
    semantics (the queue processes waits in order)."""
    cnt = 0
    for fn in nc.m.functions:
        for b in fn.blocks:
            live = b.instructions
            out = []
            changed = False
            for i in live:
                si = getattr(i, "sync_info", None)
                waits = list(si.on_wait) if si is not None else []
                movable_idx = [
                    k for k, w in enumerate(waits)
                    if getattr(w, "wait_reg", None) is None
                ]
                if len(waits) > 1 and len(movable_idx) >= 1:
                    hoist = movable_idx[:-1] if len(movable_idx) == len(waits) \
                        else movable_idx
                    hoist_set = set(hoist)
                    if len(waits) - len(hoist_set) > 1:
                        hoist_set = set(movable_idx)
                    for k in sorted(hoist_set):
                        nop = mybir.InstNoOp(name=f"I-nopw{cnt}", ins=[], outs=[])
                        cnt += 1
                        nop.engine = i.engine
                        nop.sync_info = mybir.SyncInfo(
                            on_wait=[waits[k]], on_update=[])
                        out.append(nop)
                    keep = [w for k, w in enumerate(waits) if k not in hoist_set]
                    i.sync_info = mybir.SyncInfo(
                        on_wait=keep, on_update=list(si.on_update))
                    changed = True
                out.append(i)
            if changed:
                live.clear()
                live.extend(out)
    return cnt


def _build_program(b0_nonzero: bool, b1_nonzero: bool, smb_nonzero: bool):
    nc = bass.Bass()

    emb = nc.declare_dram_parameter("emb", [V, U], BF16, isOutput=False)
    w0 = nc.declare_dram_parameter("w0", [2 * U, G4], BF16, isOutput=False)
    w1 = nc.declare_dram_parameter("w1", [2 * U, G4], BF16, isOutput=False)
    b0v = nc.declare_dram_parameter("b0v", [1, G4], BF16, isOutput=False)
    b1v = nc.declare_dram_parameter("b1v", [1, G4], BF16, isOutput=False)
    smw = nc.declare_dram_parameter("smw", [U, V], BF16, isOutput=False)
    smb = nc.declare_dram_parameter("smb", [1, V], BF16, isOutput=False)
    wvb = nc.declare_dram_parameter("wvb", [V, 264], F32, isOutput=False)
    idx_e = nc.declare_dram_parameter("idx_e", [128, NG], I32, isOutput=False)
    idx_t = nc.declare_dram_parameter("idx_t", [128, NTT], I32, isOutput=False)
    s_out = nc.declare_dram_parameter("s_out", [128, NTT], F32, isOutput=True)
    d_out = nc.declare_dram_parameter("d_out", [128, NTT], F32, isOutput=True)

    with tile.TileContext(nc) as tc:
        with (
            tc.tile_pool(name="const", bufs=1) as cpool,
            tc.tile_pool(name="wpool", bufs=1) as wpool,
            tc.tile_pool(name="persist", bufs=1) as ppool,
            tc.tile_pool(name="zpsum", bufs=2, space=bass.MemorySpace.PSUM) as zpool,
            tc.tile_pool(name="ptp", bufs=2, space=bass.MemorySpace.PSUM) as ptpool,
            tc.tile_pool(name="lpp", bufs=2, space=bass.MemorySpace.PSUM) as lppool,
        ):
            ident = cpool.tile([128, 128], BF16)
            make_identity(nc, ident[:])
            ones_row = cpool.tile([1, 128], BF16)
            nc.gpsimd.memset(ones_row[:], 1.0)

            # Weights resident in SBUF as 4 k-tiles each.
            w0t = wpool.tile([128, 4, G4], BF16)
            w1t = wpool.tile([128, 4, G4], BF16)
            for k in range(4):
                nc.sync.dma_start(w0t[:, k, :], w0[k * 128:(k + 1) * 128, :])
                nc.sync.dma_start(w1t[:, k, :], w1[k * 128:(k + 1) * 128, :])
            b0t = cpool.tile([1, G4], BF16)
            b1t = cpool.tile([1, G4], BF16)
            nc.sync.dma_start(b0t[:], b0v[:])
            nc.sync.dma_start(b1t[:], b1v[:])
            if smb_nonzero:
                smbt = cpool.tile([1, V], BF16)
                nc.sync.dma_start(smbt[:], smb[:])
            # softmax_w resident: [u-half, 16000] x 2
            smwt = wpool.tile([128, 2, V], BF16)
            for u in range(2):
                nc.sync.dma_start(smwt[:, u, :], smw[u * 128:(u + 1) * 128, :])

            idx_et = cpool.tile([128, NG], I32)
            idx_tt = cpool.tile([128, NTT], I32)
            nc.sync.dma_start(idx_et[:], idx_e[:])
            nc.sync.dma_start(idx_tt[:], idx_t[:])

            # Persistent buffers
            outt = ppool.tile([128, 2, NTOK], BF16)    # h1^T per token
            xw0 = ppool.tile([128, 8, G4], BF16)       # batched x@Wx0+b0
            sums = ppool.tile([128, NTT, NVC], F32)    # CE partial expsums
            souts = ppool.tile([128, NTT], F32)
            douts = ppool.tile([128, NTT], F32)

            with (
                tc.tile_pool(name="xg", bufs=3) as xgpool,
                tc.tile_pool(name="xt", bufs=NG) as xtpool,
                tc.tile_pool(name="gates", bufs=2) as gpool,
                tc.tile_pool(name="state", bufs=2) as spool,
                tc.tile_pool(name="hT", bufs=3) as htpool,
                tc.tile_pool(name="ce", bufs=3) as cepool,
            ):
                # ------------- gather + x-part batch precompute -------------
                xts = []
                for g in range(NG):
                    xg = xgpool.tile([128, U], BF16, tag="xg")
                    nc.gpsimd.indirect_dma_start(
                        out=xg[:],
                        out_offset=None,
                        in_=emb[:],
                        in_offset=bass.IndirectOffsetOnAxis(ap=idx_et[:, g:g + 1], axis=0),
                    )
                    xt = xtpool.tile([128, 2, 128], BF16, tag="xt")
                    xts.append(xt)
                    for u in range(2):
                        ptx = ptpool.tile([128, 128], F32, tag="pt", name="ptx")
                        nc.tensor.matmul(ptx[:], xg[:, u * 128:(u + 1) * 128],
                                         ident[:], start=True, stop=True)
                        nc.vector.tensor_copy(xt[:, u, :], ptx[:])
                    for n in range(2):
                        ns = slice(n * 512, (n + 1) * 512)
                        xp = lppool.tile([128, 512], F32, tag="lp", name="xp")
                        nc.tensor.matmul(xp[:], xt[:, 0, :], w0t[:, 0, ns],
                                         start=True, stop=False)
                        nc.tensor.matmul(xp[:], xt[:, 1, :], w0t[:, 1, ns],
                                         start=False, stop=False)
                        nc.tensor.matmul(xp[:], ones_row[0:1, :], b0t[0:1, ns],
                                         start=False, stop=True)
                        nc.vector.tensor_copy(xw0[:, g, ns], xp[:])

                # ---------------- LSTM + interleaved CE ----------------
                c_prev = None
                h0T_last = None

                def emit_group(mms, tp):
                    for i, (o, l, r) in enumerate(mms):
                        nc.tensor.matmul(
                            o, l, r,
                            start=(i == 0), stop=(i == len(mms) - 1),
                            tile_position=tp)

                # Sanitize junk partition rows (see module docstring).
                for d in range(2):
                    zz = zpool.tile([128, G4], F32, tag="z", name="zz")
                    nc.scalar.mul(zz[:, :], zz[:, :], 0.0)
                    hz = gpool.tile([128, U], BF16, tag="h", name="hz")
                    nc.gpsimd.memset(hz[:], 0.0)
                    tcz = gpool.tile([128, U], BF16, tag="tc", name="tcz")
                    nc.gpsimd.memset(tcz[:], 0.0)
                    gz = gpool.tile([128, G4], BF16, tag="G", name="gz")
                    nc.gpsimd.memset(gz[:], 0.0)
                    cz = spool.tile([128, U], BF16, tag="c", name="cz")
                    nc.gpsimd.memset(cz[:], 0.0)

                ce_units = [(tk, vc) for tk in range(NTT) for vc in range(NVC)]
                ce_done = 0

                def emit_ce_unit(tk, vc):
                    lp = lppool.tile([128, VC], F32, tag="lp", name="lp")
                    nc.tensor.matmul(lp[:], outt[:, 0, tk * 128:(tk + 1) * 128],
                                     smwt[:, 0, vc * VC:(vc + 1) * VC],
                                     start=True, stop=not smb_nonzero and False or False)
                    nc.tensor.matmul(lp[:], outt[:, 1, tk * 128:(tk + 1) * 128],
                                     smwt[:, 1, vc * VC:(vc + 1) * VC],
                                     start=False, stop=not smb_nonzero)
                    if smb_nonzero:
                        nc.tensor.matmul(lp[:], ones_row[0:1, :],
                                         smbt[0:1, vc * VC:(vc + 1) * VC],
                                         start=False, stop=True)
                    es = cepool.tile([128, VC], BF16, tag="es", name="es")
                    nc.scalar.activation(es[:], lp[:], AF.Exp)
                    nc.vector.tensor_reduce(
                        out=sums[:, tk, vc:vc + 1], in_=es[:], axis=AX.X, op=ALU.add)

                for s in range(T + 1):
                    t0 = s            # cell0 step index
                    t1 = s - 1        # cell1 step index
                    cell0 = t0 < T
                    cell1 = 0 <= t1 < T

                    z = zpool.tile([128, G4], F32, tag="z")

                    if cell0:
                        g = t0 // 16
                        c0 = (t0 % 16) * BL
                        for n in range(2):
                            ns = slice(n * 512, (n + 1) * 512)
                            mms = [(z[0:BL, ns], ident[:, c0:c0 + BL], xw0[:, g, ns])]
                            if t0 >= 1:
                                mms.append((z[0:BL, ns], h0T_last[:, 0:BL], w0t[:, 2, ns]))
                                mms.append((z[0:BL, ns], h0T_last[:, BL:2 * BL], w0t[:, 3, ns]))
                            emit_group(mms, None)

                    if cell1:
                        for n in range(2):
                            ns = slice(n * 512, (n + 1) * 512)
                            mms = [
                                (z[32:32 + BL, ns], h0T_t1c[:, 0:BL], w1t[:, 0, ns]),
                                (z[32:32 + BL, ns], h0T_t1c[:, BL:2 * BL], w1t[:, 1, ns]),
                            ]
                            if b1_nonzero:
                                mms.append((z[32:32 + BL, ns], ones_row[0:1, 0:BL], b1t[0:1, ns]))
                            elif n == 1:
                                bs = slice(512, 768)
                                mms.append((z[32:32 + BL, bs], ones_row[0:1, 0:BL], b1t[0:1, bs]))
                            if t1 >= 1:
                                mms.append((z[32:32 + BL, ns],
                                            outt[:, 0, (t1 - 1) * BL:t1 * BL], w1t[:, 2, ns]))
                                mms.append((z[32:32 + BL, ns],
                                            outt[:, 1, (t1 - 1) * BL:t1 * BL], w1t[:, 3, ns]))
                            emit_group(mms, (0, 32))

                    if cell0 and cell1:
                        lo, hi = 0, 40
                    elif cell0:
                        lo, hi = 0, BL
                    else:
                        lo, hi = 32, 40

                    # gates: one tanh over all 1024 cols (i,o,f pre-scaled by
                    # 1/2 in the weights; j plain)
                    tg = gpool.tile([128, G4], BF16, tag="G", name="tg")
                    nc.scalar.activation(tg[lo:hi, :], z[lo:hi, :], AF.Tanh)

                    # c' = 2c recurrence:
                    # c'_new = c' * (1+tf)/2 + (1+ti) * tj
                    s2 = spool.tile([128, U], BF16, tag="s2", name="s2")
                    nc.gpsimd.tensor_scalar_add(s2[lo:hi, :], tg[lo:hi, _TI], 1.0)
                    m2 = spool.tile([128, U], BF16, tag="m2", name="m2")
                    nc.vector.tensor_tensor(
                        out=m2[lo:hi, :], in0=s2[lo:hi, :], in1=tg[lo:hi, _TJ],
                        op=ALU.mult)
                    cp_new = spool.tile([128, U], BF16, tag="c", name="cp_new")
                    if s == 0:
                        nc.vector.tensor_copy(cp_new[lo:hi, :], m2[lo:hi, :])
                        nc.gpsimd.memset(cp_new[32:40, :], 0.0)
                    else:
                        s1 = spool.tile([128, U], BF16, tag="s1", name="s1")
                        nc.gpsimd.tensor_scalar(
                            s1[lo:hi, :], tg[lo:hi, _TF], 1.0, 0.5,
                            op0=ALU.add, op1=ALU.mult)
                        m1 = spool.tile([128, U], BF16, tag="m1", name="m1")
                        nc.vector.tensor_tensor(
                            out=m1[lo:hi, :], in0=s1[lo:hi, :], in1=c_prev[lo:hi, :],
                            op=ALU.mult)
                        nc.vector.tensor_tensor(
                            out=cp_new[lo:hi, :], in0=m1[lo:hi, :], in1=m2[lo:hi, :],
                            op=ALU.add)

                    # h = tanh(c'/2) * (1+to)/2
                    tc_t = gpool.tile([128, U], BF16, tag="tc", name="tc_t")
                    nc.scalar.activation(tc_t[lo:hi, :], cp_new[lo:hi, :], AF.Tanh,
                                         scale=0.5)
                    s3 = spool.tile([128, U], BF16, tag="s3", name="s3")
                    nc.gpsimd.tensor_scalar(
                        s3[lo:hi, :], tg[lo:hi, _TO], 1.0, 0.5,
                        op0=ALU.add, op1=ALU.mult)
                    h_t = gpool.tile([128, U], BF16, tag="h", name="h_t")
                    nc.vector.tensor_tensor(
                        out=h_t[lo:hi, :], in0=tc_t[lo:hi, :], in1=s3[lo:hi, :],
                        op=ALU.mult)

                    h0T_t = None
                    if cell0:
                        h0T_t = htpool.tile([128, 2 * BL], BF16, tag="h0T", name="h0T_t")
                    for u in range(2):
                        pt = ptpool.tile([128, 128], F32, tag="pt", name="pt")
                        nc.tensor.matmul(pt[:], h_t[:, u * 128:(u + 1) * 128], ident[:],
                                         start=True, stop=True)
                        if cell0:
                            nc.vector.tensor_copy(h0T_t[:, u * BL:(u + 1) * BL], pt[:, 0:BL])
                        if cell1:
                            nc.vector.tensor_copy(
                                outt[:, u, t1 * BL:(t1 + 1) * BL], pt[:, 32:40])

                    h0T_t1c = h0T_last if not cell0 else h0T_t
                    if cell0:
                        h0T_last = h0T_t
                    c_prev = cp_new

                    # interleaved CE: chunk tk is complete after iteration
                    # s = 16*tk + 16 (outt cols for t1 = 16tk+15 written above)
                    ready = min(NTT * NVC, NVC * max(0, (s - 16) // 16 + 1))
                    target = min(NTT * NVC, max(0, (s - 16) * (NTT * NVC) // 110))
                    while ce_done < min(ready, target):
                        tk, vc = ce_units[ce_done]
                        emit_ce_unit(tk, vc)
                        ce_done += 1

                while ce_done < NTT * NVC:
                    tk, vc = ce_units[ce_done]
                    emit_ce_unit(tk, vc)
                    ce_done += 1

                # ---------------- CE epilogue ----------------
                for tk in range(NTT):
                    nc.vector.tensor_reduce(
                        out=souts[:, tk:tk + 1], in_=sums[:, tk, :], axis=AX.X,
                        op=ALU.add)
                    wtg = cepool.tile([128, 264], F32, tag="wtg", name="wtg")
                    nc.gpsimd.indirect_dma_start(
                        out=wtg[:],
                        out_offset=None,
                        in_=wvb[:],
                        in_offset=bass.IndirectOffsetOnAxis(ap=idx_tt[:, tk:tk + 1], axis=0),
                    )
                    outb = cepool.tile([128, U], F32, tag="outb", name="outb")
                    for u in range(2):
                        ptc = ptpool.tile([128, 128], F32, tag="pt", name="ptc")
                        nc.tensor.matmul(
                            ptc[:], outt[:, u, tk * 128:(tk + 1) * 128], ident[:],
                            start=True, stop=True)
                        nc.vector.tensor_copy(outb[:, u * 128:(u + 1) * 128], ptc[:])
                    ttr = cepool.tile([128, U], F32, tag="ttr", name="ttr")
                    nc.vector.tensor_tensor(
                        out=ttr[:], in0=outb[:], in1=wtg[:, 0:U], op=ALU.mult)
                    dtmp = cepool.tile([128, 1], F32, tag="dtmp", name="dtmp")
                    nc.vector.tensor_reduce(
                        out=dtmp[:], in_=ttr[:], axis=AX.X, op=ALU.add)
                    nc.vector.tensor_tensor(
                        out=douts[:, tk:tk + 1], in0=dtmp[:], in1=wtg[:, U:U + 1],
                        op=ALU.add)

                nc.sync.dma_start(s_out[:], souts[:])
                nc.sync.dma_start(d_out[:], douts[:])

    _split_multiwaits(nc)
    return nc


def _get_program(flags):
    if flags not in _CACHE:
        _CACHE[flags] = _build_program(*flags)
    return _CACHE[flags]


def _prep_host(input_data, targets, embedding, W0, b0, W1, b1, softmax_w, softmax_b):
    """Host-side layout prep: gate permutation to [i|o|f|j], the
    sigmoid-via-tanh 1/2 pre-scaling of the i/o/f columns, forget bias,
    dtype casts, and per-core index arrays."""
    W0p = np.ascontiguousarray(np.asarray(W0, np.float32)[:, GPERM])
    W1p = np.ascontiguousarray(np.asarray(W1, np.float32)[:, GPERM])
    b0p = np.asarray(b0, np.float32)[GPERM].copy()
    b1p = np.asarray(b1, np.float32)[GPERM].copy()
    b0_nonzero = bool(np.any(b0p))
    b1_nonzero = bool(np.any(b1p))
    smb = np.asarray(softmax_b, np.float32)
    smb_nonzero = bool(np.any(smb))

    # forget bias, then scale i/o/f (cols 0:768) by 1/2 for tanh-sigmoid
    b0e = b0p.copy()
    b0e[_TF] += FORGET_BIAS
    b1e = b1p.copy()
    b1e[_TF] += FORGET_BIAS
    W0p[:, 0:3 * U] *= 0.5
    W1p[:, 0:3 * U] *= 0.5
    b0e[0:3 * U] *= 0.5
    b1e[0:3 * U] *= 0.5

    wvb = np.zeros((V, 264), np.float32)
    wvb[:, 0:U] = np.asarray(softmax_w, np.float32).T
    wvb[:, U] = smb

    bf = ml_dtypes.bfloat16
    shared = {
        "emb": np.ascontiguousarray(np.asarray(embedding, np.float32)).astype(bf),
        "w0": W0p.astype(bf),
        "w1": W1p.astype(bf),
        "b0v": b0e[None, :].astype(bf),
        "b1v": b1e[None, :].astype(bf),
        "smw": np.ascontiguousarray(np.asarray(softmax_w, np.float32)).astype(bf),
        "smb": smb[None, :].astype(bf),
        "wvb": wvb,
    }
    in_maps = []
    ids = np.asarray(input_data, np.int32)
    tgs = np.asarray(targets, np.int32)
    for c in range(NCORES):
        tok_e = ids[c * BL:(c + 1) * BL, :].T.reshape(-1)   # t-major [1024]
        tok_t = tgs[c * BL:(c + 1) * BL, :].T.reshape(-1)
        m = dict(shared)
        m["idx_e"] = np.ascontiguousarray(tok_e.reshape(NG, 128).T)
        m["idx_t"] = np.ascontiguousarray(tok_t.reshape(NTT, 128).T)
        in_maps.append(m)
    return (b0_nonzero, b1_nonzero, smb_nonzero), in_maps


def run(trace=False, **inputs):
    flags, in_maps = _prep_host(**inputs)
    nc = _get_program(flags)
    res = run_bass_kernel_spmd(nc, in_maps, list(range(NCORES)), trace=trace)
    total = 0.0
    for r in res.results:
        s = r["s_out"].astype(np.float64)
        dd = r["d_out"].astype(np.float64)
        total += float(np.sum(np.log(s) - dd))
    cost = np.float32(total / (B * T))
    return cost, res


def kernel(**inputs):
    cost, _ = run(trace=False, **inputs)
    return cost


# revision 22
# speedup vs baseline: 1.4793x; 1.4793x over previous
"""CharRNN (2-layer LSTM + softmax CE) Trainium2 Bass kernel.

Sharding: data-parallel over batch (B=64 -> 8 rows/core on 8 cores).
Each core runs the full T=128 recurrence for its 8 sequences and the
cross-entropy over its own 1024 tokens; host sums the 8 partial NLLs
(final per-token ln() + reduction also on host — 8KB/core of output).

Device design (per core):
  - Interleaved cells: cell0(step s) at partitions 0:8 and cell1(step
    s-1) at partitions 32:40 share every instruction (distinct PE column
    groups let their matmuls overlap on the array; ACT/DVE cost is
    free-dim bound so the packing halves instruction count).
  - All gates via tanh only (sigmoid(x) = (1+tanh(x/2))/2, with the /2
    folded into the i/o/f weight columns on the host and the cell state
    kept as c' = 2c). One activation table set (exp_and_others holds
    tanh AND exp) so cross-entropy work interleaves with the recurrence
    with no table thrash.
  - z = [x,h] @ W with x_h^T as stationary operand and W streamed
    (weights re-stream each step; batch is tiny so this is the cheap
    direction). The x@Wx0+b0 part is batched over all tokens up front
    into an SBUF buffer and injected per-step with an identity-selector
    matmul (halves the per-step x-part stream).
  - h transposed each step via identity matmul (PSUM junk rows are
    zeroed once: NaN junk would poison the product since NaN*0=NaN).
  - CE: softmax_w lives in SBUF (8MB bf16); logits [128 tok, 500 vocab]
    chunks are computed/exp'ed/summed *inside* the step loop (2-3 chunks
    per step once their tokens exist) to fill PE gaps and keep the HAM
    clock warm. Target logit via indirect row gather of host-transposed
    softmax_w and a multiply+reduce.
"""

import sys

for _p in ("/opt/trn_rl_repo",):
    if _p not in sys.path:
        sys.path.insert(0, _p)

import ml_dtypes
import numpy as np

import concourse.bass as bass
import concourse.mybir as mybir
import concourse.tile as tile
from concourse.bass_utils import run_bass_kernel_spmd
from concourse.masks import make_identity

F32 = mybir.dt.float32
BF16 = mybir.dt.bfloat16
I32 = mybir.dt.int32
AF = mybir.ActivationFunctionType
ALU = mybir.AluOpType
AX = mybir.AxisListType

# Problem shapes (hardcoded per contest rules).
V, B, T, U = 16000, 64, 128, 256
NCORES = 8
BL = B // NCORES            # 8 batch rows per core
NTOK = BL * T               # 1024 tokens per core
G4 = 4 * U                  # 1024 gate width
NG = 8                      # embedding gathers per core (128 tokens each)
VC = 500                    # vocab chunk for CE
NVC = V // VC               # 32 chunks
NTT = NTOK // 128           # 8 token tiles for CE

# Gate permutation: reference z columns are [i|j|f|o]; we reorder to
# [i|o|f|j]. Slices in permuted space (all of i,o,f get tanh(x/2) with
# the 1/2 folded into the weights; j gets plain tanh):
_TI = slice(0 * U, 1 * U)
_TO = slice(1 * U, 2 * U)
_TF = slice(2 * U, 3 * U)
_TJ = slice(3 * U, 4 * U)
GPERM = np.r_[0:U, 3 * U:4 * U, 2 * U:3 * U, U:2 * U]

FORGET_BIAS = 1.0

_CACHE = {}


def _split_multiwaits(nc):
    """Walrus codegen supports only ONE semaphore wait per HW instruction
    (NEURON_ISA_TPB_EVENTS has a single wait slot) and errors out on
    instructions carrying more. Tile emits multi-wait sync_info freely, so
    split: for an instruction with k>1 waits, insert k-1 NoOps on the same
    engine queue immediately before it, each carrying one wait. Identical
    semantics (the queue processes waits in order)."""
    cnt = 0
    for fn in nc.m.functions:
        for b in fn.blocks:
            live = b.instructions
            out = []
            changed = False
            for i in live:
                si = getattr(i, "sync_info", None)
                waits = list(si.on_wait) if si is not None else []
                movable_idx = [
                    k for k, w in enumerate(waits)
                    if getattr(w, "wait_reg", None) is None
                ]
                if len(waits) > 1 and len(movable_idx) >= 1:
                    hoist = movable_idx[:-1] if len(movable_idx) == len(waits) \
                        else movable_idx
                    hoist_set = set(hoist)
                    if len(waits) - len(hoist_set) > 1:
                        hoist_set = set(movable_idx)
                    for k in sorted(hoist_set):
                        nop = mybir.InstNoOp(name=f"I-nopw{cnt}", ins=[], outs=[])
                        cnt += 1
                        nop.engine = i.engine
                        nop.sync_info = mybir.SyncInfo(
                            on_wait=[waits[k]], on_update=[])
                        out.append(nop)
                    keep = [w for k, w in enumerate(waits) if k not in hoist_set]
                    i.sync_info = mybir.SyncInfo(
                        on_wait=keep, on_update=list(si.on_update))
                    changed = True
                out.append(i)
            if changed:
                live.clear()
                live.extend(out)
    return cnt


def _build_program(b0_nonzero: bool, b1_nonzero: bool, smb_nonzero: bool):
    nc = bass.Bass()

    emb = nc.declare_dram_parameter("emb", [V, U], BF16, isOutput=False)
    w0 = nc.declare_dram_parameter("w0", [2 * U, G4], BF16, isOutput=False)
    w1 = nc.declare_dram_parameter("w1", [2 * U, G4], BF16, isOutput=False)
    b0v = nc.declare_dram_parameter("b0v", [1, G4], BF16, isOutput=False)
    b1v = nc.declare_dram_parameter("b1v", [1, G4], BF16, isOutput=False)
    smw = nc.declare_dram_parameter("smw", [U, V], BF16, isOutput=False)
    smb = nc.declare_dram_parameter("smb", [1, V], BF16, isOutput=False)
    wvb = nc.declare_dram_parameter("wvb", [V, 264], F32, isOutput=False)
    idx_e = nc.declare_dram_parameter("idx_e", [128, NG], I32, isOutput=False)
    idx_t = nc.declare_dram_parameter("idx_t", [128, NTT], I32, isOutput=False)
    s_out = nc.declare_dram_parameter("s_out", [128, NTT], F32, isOutput=True)
    d_out = nc.declare_dram_parameter("d_out", [128, NTT], F32, isOutput=True)

    with tile.TileContext(nc) as tc:
        with (
            tc.tile_pool(name="const", bufs=1) as cpool,
            tc.tile_pool(name="wpool", bufs=1) as wpool,
            tc.tile_pool(name="persist", bufs=1) as ppool,
            tc.tile_pool(name="zpsum", bufs=2, space=bass.MemorySpace.PSUM) as zpool,
            tc.tile_pool(name="ptp", bufs=2, space=bass.MemorySpace.PSUM) as ptpool,
            tc.tile_pool(name="lpp", bufs=2, space=bass.MemorySpace.PSUM) as lppool,
        ):
            ident = cpool.tile([128, 128], BF16)
            make_identity(nc, ident[:])
            ones_row = cpool.tile([1, 128], BF16)
            nc.gpsimd.memset(ones_row[:], 1.0)

            # Weights resident in SBUF as 4 k-tiles each.
            w0t = wpool.tile([128, 4, G4], BF16)
            w1t = wpool.tile([128, 4, G4], BF16)
            for k in range(4):
                nc.sync.dma_start(w0t[:, k, :], w0[k * 128:(k + 1) * 128, :])
                nc.sync.dma_start(w1t[:, k, :], w1[k * 128:(k + 1) * 128, :])
            b0t = cpool.tile([1, G4], BF16)
            b1t = cpool.tile([1, G4], BF16)
            nc.sync.dma_start(b0t[:], b0v[:])
            nc.sync.dma_start(b1t[:], b1v[:])
            if smb_nonzero:
                smbt = cpool.tile([1, V], BF16)
                nc.sync.dma_start(smbt[:], smb[:])
            # softmax_w resident: [u-half, 16000] x 2
            smwt = wpool.tile([128, 2, V], BF16)
            for u in range(2):
                nc.sync.dma_start(smwt[:, u, :], smw[u * 128:(u + 1) * 128, :])

            idx_et = cpool.tile([128, NG], I32)
            idx_tt = cpool.tile([128, NTT], I32)
            nc.sync.dma_start(idx_et[:], idx_e[:])
            nc.sync.dma_start(idx_tt[:], idx_t[:])

            # Persistent buffers
            outt = ppool.tile([128, 2, NTOK], BF16)    # h1^T per token
            xw0 = ppool.tile([128, 8, G4], BF16)       # batched x@Wx0+b0
            sums = ppool.tile([128, NTT, NVC], F32)    # CE partial expsums
            souts = ppool.tile([128, NTT], F32)
            douts = ppool.tile([128, NTT], F32)

            with (
                tc.tile_pool(name="xg", bufs=3) as xgpool,
                tc.tile_pool(name="xt", bufs=NG) as xtpool,
                tc.tile_pool(name="gates", bufs=2) as gpool,
                tc.tile_pool(name="state", bufs=2) as spool,
                tc.tile_pool(name="hT", bufs=3) as htpool,
                tc.tile_pool(name="ce", bufs=3) as cepool,
            ):
                # ------------- gather + x-part batch precompute -------------
                xts = []
                for g in range(NG):
                    xg = xgpool.tile([128, U], BF16, tag="xg")
                    nc.gpsimd.indirect_dma_start(
                        out=xg[:],
                        out_offset=None,
                        in_=emb[:],
                        in_offset=bass.IndirectOffsetOnAxis(ap=idx_et[:, g:g + 1], axis=0),
                    )
                    xt = xtpool.tile([128, 2, 128], BF16, tag="xt")
                    xts.append(xt)
                    for u in range(2):
                        ptx = ptpool.tile([128, 128], F32, tag="pt", name="ptx")
                        nc.tensor.matmul(ptx[:], xg[:, u * 128:(u + 1) * 128],
                                         ident[:], start=True, stop=True)
                        nc.vector.tensor_copy(xt[:, u, :], ptx[:])
                    for n in range(2):
                        ns = slice(n * 512, (n + 1) * 512)
                        xp = lppool.tile([128, 512], F32, tag="lp", name="xp")
                        nc.tensor.matmul(xp[:], xt[:, 0, :], w0t[:, 0, ns],
                                         start=True, stop=False)
                        nc.tensor.matmul(xp[:], xt[:, 1, :], w0t[:, 1, ns],
                                         start=False, stop=False)
                        nc.tensor.matmul(xp[:], ones_row[0:1, :], b0t[0:1, ns],
                                         start=False, stop=True)
                        nc.vector.tensor_copy(xw0[:, g, ns], xp[:])

                # ---------------- LSTM + interleaved CE ----------------
                c_prev = None
                h0T_last = None

                def emit_group(mms, tp):
                    for i, (o, l, r) in enumerate(mms):
                        nc.tensor.matmul(
                            o, l, r,
                            start=(i == 0), stop=(i == len(mms) - 1),
                            tile_position=tp)

                # Sanitize junk partition rows (see module docstring).
                for d in range(2):
                    zz = zpool.tile([128, G4], F32, tag="z", name="zz")
                    nc.scalar.mul(zz[:, :], zz[:, :], 0.0)
                    hz = gpool.tile([128, U], BF16, tag="h", name="hz")
                    nc.gpsimd.memset(hz[:], 0.0)
                    tcz = gpool.tile([128, U], BF16, tag="tc", name="tcz")
                    nc.gpsimd.memset(tcz[:], 0.0)
                    gz = gpool.tile([128, G4], BF16, tag="G", name="gz")
                    nc.gpsimd.memset(gz[:], 0.0)
                    cz = spool.tile([128, U], BF16, tag="c", name="cz")
                    nc.gpsimd.memset(cz[:], 0.0)

                ce_units = [(tk, vc) for tk in range(NTT) for vc in range(NVC)]
                ce_done = 0

                def emit_ce_unit(tk, vc):
                    lp = lppool.tile([128, VC], F32, tag="lp", name="lp")
                    nc.tensor.matmul(lp[:], outt[:, 0, tk * 128:(tk + 1) * 128],
                                     smwt[:, 0, vc * VC:(vc + 1) * VC],
                                     start=True, stop=not smb_nonzero and False or False)
                    nc.tensor.matmul(lp[:], outt[:, 1, tk * 128:(tk + 1) * 128],
                                     smwt[:, 1, vc * VC:(vc + 1) * VC],
                                     start=False, stop=not smb_nonzero)
                    if smb_nonzero:
                        nc.tensor.matmul(lp[:], ones_row[0:1, :],
                                         smbt[0:1, vc * VC:(vc + 1) * VC],
                                         start=False, stop=True)
                    es = cepool.tile([128, VC], BF16, tag="es", name="es")
                    nc.scalar.activation(es[:], lp[:], AF.Exp)
                    nc.vector.tensor_reduce(
                        out=sums[:, tk, vc:vc + 1], in_=es[:], axis=AX.X, op=ALU.add)

                for s in range(T + 1):
                    t0 = s            # cell0 step index
                    t1 = s - 1        # cell1 step index
                    cell0 = t0 < T
                    cell1 = 0 <= t1 < T

                    z = zpool.tile([128, G4], F32, tag="z")

                    if cell0:
                        g = t0 // 16
                        c0 = (t0 % 16) * BL
                        for n in range(2):
                            ns = slice(n * 512, (n + 1) * 512)
                            mms = [(z[0:BL, ns], ident[:, c0:c0 + BL], xw0[:, g, ns])]
                            if t0 >= 1:
                                mms.append((z[0:BL, ns], h0T_last[:, 0:BL], w0t[:, 2, ns]))
                                mms.append((z[0:BL, ns], h0T_last[:, BL:2 * BL], w0t[:, 3, ns]))
                            emit_group(mms, None)

                    if cell1:
                        for n in range(2):
                            ns = slice(n * 512, (n + 1) * 512)
                            mms = [
                                (z[32:32 + BL, ns], h0T_t1c[:, 0:BL], w1t[:, 0, ns]),
                                (z[32:32 + BL, ns], h0T_t1c[:, BL:2 * BL], w1t[:, 1, ns]),
                            ]
                            if b1_nonzero:
                                mms.append((z[32:32 + BL, ns], ones_row[0:1, 0:BL], b1t[0:1, ns]))
                            elif n == 1:
                                bs = slice(512, 768)
                                mms.append((z[32:32 + BL, bs], ones_row[0:1, 0:BL], b1t[0:1, bs]))
                            if t1 >= 1:
                                mms.append((z[32:32 + BL, ns],
                                            outt[:, 0, (t1 - 1) * BL:t1 * BL], w1t[:, 2, ns]))
                                mms.append((z[32:32 + BL, ns],
                                            outt[:, 1, (t1 - 1) * BL:t1 * BL], w1t[:, 3, ns]))
                            emit_group(mms, (0, 32))

                    if cell0 and cell1:
                        lo, hi = 0, 40
                    elif cell0:
                        lo, hi = 0, BL
                    else:
                        lo, hi = 32, 40

                    # gates: one tanh over all 1024 cols (i,o,f pre-scaled by
                    # 1/2 in the weights; j plain)
                    tg = gpool.tile([128, G4], BF16, tag="G", name="tg")
                    nc.scalar.activation(tg[lo:hi, :], z[lo:hi, :], AF.Tanh)

                    # c' = 2c recurrence:
                    # c'_new = c' * (1+tf)/2 + (1+ti) * tj
                    s2 = spool.tile([128, U], BF16, tag="s2", name="s2")
                    nc.vector.tensor_scalar_add(s2[lo:hi, :], tg[lo:hi, _TI], 1.0)
                    m2 = spool.tile([128, U], BF16, tag="m2", name="m2")
                    nc.vector.tensor_tensor(
                        out=m2[lo:hi, :], in0=s2[lo:hi, :], in1=tg[lo:hi, _TJ],
                        op=ALU.mult)
                    cp_new = spool.tile([128, U], BF16, tag="c", name="cp_new")
                    if s == 0:
                        nc.vector.tensor_copy(cp_new[lo:hi, :], m2[lo:hi, :])
                        nc.gpsimd.memset(cp_new[32:40, :], 0.0)
                    else:
                        s1 = spool.tile([128, U], BF16, tag="s1", name="s1")
                        nc.vector.tensor_scalar(
                            s1[lo:hi, :], tg[lo:hi, _TF], 1.0, 0.5,
                            op0=ALU.add, op1=ALU.mult)
                        m1 = spool.tile([128, U], BF16, tag="m1", name="m1")
                        nc.vector.tensor_tensor(
                            out=m1[lo:hi, :], in0=s1[lo:hi, :], in1=c_prev[lo:hi, :],
                            op=ALU.mult)
                        nc.vector.tensor_tensor(
                            out=cp_new[lo:hi, :], in0=m1[lo:hi, :], in1=m2[lo:hi, :],
                            op=ALU.add)

                    # h = tanh(c'/2) * (1+to)/2
                    tc_t = gpool.tile([128, U], BF16, tag="tc", name="tc_t")
                    nc.scalar.activation(tc_t[lo:hi, :], cp_new[lo:hi, :], AF.Tanh,
                                         scale=0.5)
                    s3 = spool.tile([128, U], BF16, tag="s3", name="s3")
                    nc.vector.tensor_scalar(
                        s3[lo:hi, :], tg[lo:hi, _TO], 1.0, 0.5,
                        op0=ALU.add, op1=ALU.mult)
                    h_t = gpool.tile([128, U], BF16, tag="h", name="h_t")
                    nc.vector.tensor_tensor(
                        out=h_t[lo:hi, :], in0=tc_t[lo:hi, :], in1=s3[lo:hi, :],
                        op=ALU.mult)

                    h0T_t = None
                    if cell0:
                        h0T_t = htpool.tile([128, 2 * BL], BF16, tag="h0T", name="h0T_t")
                    for u in range(2):
                        pt = ptpool.tile([128, 128], F32, tag="pt", name="pt")
                        nc.tensor.matmul(pt[:], h_t[:, u * 128:(u + 1) * 128], ident[:],
                                         start=True, stop=True)
                        if cell0:
                            nc.vector.tensor_copy(h0T_t[:, u * BL:(u + 1) * BL], pt[:, 0:BL])
                        if cell1:
                            nc.vector.tensor_copy(
                                outt[:, u, t1 * BL:(t1 + 1) * BL], pt[:, 32:40])

                    h0T_t1c = h0T_last if not cell0 else h0T_t
                    if cell0:
                        h0T_last = h0T_t
                    c_prev = cp_new

                    # interleaved CE: chunk tk is complete after iteration
                    # s = 16*tk + 16 (outt cols for t1 = 16tk+15 written above)
                    ready = min(NTT * NVC, NVC * max(0, (s - 16) // 16 + 1))
                    target = min(NTT * NVC, max(0, (s - 16) * (NTT * NVC) // 110))
                    while ce_done < min(ready, target):
                        tk, vc = ce_units[ce_done]
                        emit_ce_unit(tk, vc)
                        ce_done += 1

                while ce_done < NTT * NVC:
                    tk, vc = ce_units[ce_done]
                    emit_ce_unit(tk, vc)
                    ce_done += 1

                # ---------------- CE epilogue ----------------
                for tk in range(NTT):
                    nc.vector.tensor_reduce(
                        out=souts[:, tk:tk + 1], in_=sums[:, tk, :], axis=AX.X,
                        op=ALU.add)
                    wtg = cepool.tile([128, 264], F32, tag="wtg", name="wtg")
                    nc.gpsimd.indirect_dma_start(
                        out=wtg[:],
                        out_offset=None,
                        in_=wvb[:],
                        in_offset=bass.IndirectOffsetOnAxis(ap=idx_tt[:, tk:tk + 1], axis=0),
                    )
                    outb = cepool.tile([128, U], F32, tag="outb", name="outb")
                    for u in range(2):
                        ptc = ptpool.tile([128, 128], F32, tag="pt", name="ptc")
                        nc.tensor.matmul(
                            ptc[:], outt[:, u, tk * 128:(tk + 1) * 128], ident[:],
                            start=True, stop=True)
                        nc.vector.tensor_copy(outb[:, u * 128:(u + 1) * 128], ptc[:])
                    ttr = cepool.tile([128, U], F32, tag="ttr", name="ttr")
                    nc.vector.tensor_tensor(
                        out=ttr[:], in0=outb[:], in1=wtg[:, 0:U], op=ALU.mult)
                    dtmp = cepool.tile([128, 1], F32, tag="dtmp", name="dtmp")
                    nc.vector.tensor_reduce(
                        out=dtmp[:], in_=ttr[:], axis=AX.X, op=ALU.add)
                    nc.vector.tensor_tensor(
                        out=douts[:, tk:tk + 1], in0=dtmp[:], in1=wtg[:, U:U + 1],
                        op=ALU.add)

                nc.sync.dma_start(s_out[:], souts[:])
                nc.sync.dma_start(d_out[:], douts[:])

    _split_multiwaits(nc)
    return nc


def _get_program(flags):
    if flags not in _CACHE:
        _CACHE[flags] = _build_program(*flags)
    return _CACHE[flags]


def _prep_host(input_data, targets, embedding, W0, b0, W1, b1, softmax_w, softmax_b):
    """Host-side layout prep: gate permutation to [i|o|f|j], the
    sigmoid-via-tanh 1/2 pre-scaling of the i/o/f columns, forget bias,
    dtype casts, and per-core index arrays."""
    W0p = np.ascontiguousarray(np.asarray(W0, np.float32)[:, GPERM])
    W1p = np.ascontiguousarray(np.asarray(W1, np.float32)[:, GPERM])
    b0p = np.asarray(b0, np.float32)[GPERM].copy()
    b1p = np.asarray(b1, np.float32)[GPERM].copy()
    b0_nonzero = bool(np.any(b0p))
    b1_nonzero = bool(np.any(b1p))
    smb = np.asarray(softmax_b, np.float32)
    smb_nonzero = bool(np.any(smb))

    # forget bias, then scale i/o/f (cols 0:768) by 1/2 for tanh-sigmoid
    b0e = b0p.copy()
    b0e[_TF] += FORGET_BIAS
    b1e = b1p.copy()
    b1e[_TF] += FORGET_BIAS
    W0p[:, 0:3 * U] *= 0.5
    W1p[:, 0:3 * U] *= 0.5
    b0e[0:3 * U] *= 0.5
    b1e[0:3 * U] *= 0.5

    wvb = np.zeros((V, 264), np.float32)
    wvb[:, 0:U] = np.asarray(softmax_w, np.float32).T
    wvb[:, U] = smb

    bf = ml_dtypes.bfloat16
    shared = {
        "emb": np.ascontiguousarray(np.asarray(embedding, np.float32)).astype(bf),
        "w0": W0p.astype(bf),
        "w1": W1p.astype(bf),
        "b0v": b0e[None, :].astype(bf),
        "b1v": b1e[None, :].astype(bf),
        "smw": np.ascontiguousarray(np.asarray(softmax_w, np.float32)).astype(bf),
        "smb": smb[None, :].astype(bf),
        "wvb": wvb,
    }
    in_maps = []
    ids = np.asarray(input_data, np.int32)
    tgs = np.asarray(targets, np.int32)
    for c in range(NCORES):
        tok_e = ids[c * BL:(c + 1) * BL, :].T.reshape(-1)   # t-major [1024]
        tok_t = tgs[c * BL:(c + 1) * BL, :].T.reshape(-1)
        m = dict(shared)
        m["idx_e"] = np.ascontiguousarray(tok_e.reshape(NG, 128).T)
        m["idx_t"] = np.ascontiguousarray(tok_t.reshape(NTT, 128).T)
        in_maps.append(m)
    return (b0_nonzero, b1_nonzero, smb_nonzero), in_maps


def run(trace=False, **inputs):
    flags, in_maps = _prep_host(**inputs)
    nc = _get_program(flags)
    res = run_bass_kernel_spmd(nc, in_maps, list(range(NCORES)), trace=trace)
    total = 0.0
    for r in res.results:
        s = r["s_out"].astype(np.float64)
        dd = r["d_out"].astype(np.float64)
        total += float(np.sum(np.log(s) - dd))
    cost = np.float32(total / (B * T))
    return cost, res


def kernel(**inputs):
    cost, _ = run(trace=False, **inputs)
    return cost
